# revision 1
# baseline (speedup 1.0000x reference)
"""GQA forward kernel for 8 Trainium2 NeuronCores.

Problem: B=2, T=2048, C=2048, 32 Q heads / 8 KV heads, head_dim=64, causal.

Sharding: 2-way data parallel over batch x 4-way tensor parallel over KV-head
pairs. Each core handles one batch element and 2 KV heads (8 Q heads), computes
its slice of Q/K/V projections, causal attention, and a partial output
projection (transposed). Host sums the 4 partials per batch and adds bo.

Design (v5):
  - All matmul inputs bf16 (host converts); PSUM accumulation stays f32.
  - Attention is a single software-pipelined stripe stream per q-slice:
    scores run one stripe ahead of AV (covering the exp latency), continuing
    across head-pair (j) boundaries; PE filler units (next slice's
    projections, previous slice's output projection) are interleaved between
    stripes so the PE never waits on the ACT-bound exp stream.
  - PSUM: tag "st" 2x [128,1024] 2-bank slots (score pairs + normalization
    broadcast), tag "aux" 2x 1-bank slots (projection/oproj sub-GEMMs,
    V-transposes), av_a/av_b accumulator banks. Separate tags have separate
    allocation FIFOs so filler GEMMs schedule concurrently with stripes.
  - Scores for the two KV heads go into one [128,1024] 2-bank slot -> single
    exp Activation per stripe; the two K=64 score matmuls row-pack in the PE
    array (tile_position (0,0)/(64,0) derived from base partitions).
  - Softmax denominators via ones-column appended to V (rows 64/129 of v_s);
    normalization = 2 DVE row copies + 1 DVE reciprocal + 1 PE broadcast
    matmul + 2 DVE muls per (qs,j) -- no gpsimd.
  - Diagonal-stripe matmuls restrict their moving range to the unmasked
    columns; exp runs on the full [128,1024] tile (masked cols are zeroed by
    the PSUM bank clear, exp(0)=1 junk is never read by the restricted AV).
  - DMA batching: one strided transfer per xt t-slice, per weight tensor,
    per opt q-slice store, and one packed bias tile (HWDGE ring is serial
    per transfer, so fewer/bigger transfers).
"""

import sys
import numpy as np

T = 2048
C = 2048
D = 64
NT = 512          # t/q slice width (matmul moving free dim)
TS = T // NT      # 4 slices
KT = C // 128     # 16 contraction tiles
QC = 4            # local q-col tiles of 128 (512 local q cols)

_CACHE = {}


def _ensure_path():
    for p in ("/opt/trn_rl_repo",):
        if p not in sys.path:
            sys.path.insert(0, p)


def _build(reps=1, sections=("p1", "attn", "oproj")):
    sections = tuple(sections)
    key = (reps, sections)
    if key in _CACHE:
        return _CACHE[key]
    _ensure_path()
    import concourse.mybir as mybir
    import concourse.bacc as bacc
    from concourse import tile
    from concourse.masks import make_identity
    from contextlib import ExitStack

    dt = mybir.dt
    f32 = dt.float32
    bf16 = dt.bfloat16
    AF = mybir.ActivationFunctionType

    nc = bacc.Bacc(None, target_bir_lowering=False)
    xt_d = nc.declare_dram_parameter("xt", (C, T), bf16, isOutput=False)
    wq_d = nc.declare_dram_parameter("wq", (C, 512), bf16, isOutput=False)
    wk_d = nc.declare_dram_parameter("wk", (C, 128), bf16, isOutput=False)
    wv_d = nc.declare_dram_parameter("wv", (C, 128), bf16, isOutput=False)
    wo_d = nc.declare_dram_parameter("wo", (512, C), bf16, isOutput=False)
    bias_d = nc.declare_dram_parameter("bias", (128, 6), f32, isOutput=False)
    tri_d = nc.declare_dram_parameter("tri", (128, 128), bf16, isOutput=False)
    opt_d = nc.declare_dram_parameter("opt", (C, T), bf16, isOutput=True)

    with tile.TileContext(nc) as tc, ExitStack() as ctx:
        constp = ctx.enter_context(tc.tile_pool(name="const", bufs=1))
        wp = ctx.enter_context(tc.tile_pool(name="w", bufs=1))
        pers = ctx.enter_context(tc.tile_pool(name="pers", bufs=1))
        otp = ctx.enter_context(tc.tile_pool(name="ot", bufs=2))
        xtp = ctx.enter_context(tc.tile_pool(name="xt", bufs=2))
        tmpp = ctx.enter_context(tc.tile_pool(name="tmp", bufs=2))
        ptp = ctx.enter_context(tc.tile_pool(name="pt", bufs=6))
        rcp = ctx.enter_context(tc.tile_pool(name="rc", bufs=2))
        osp = ctx.enter_context(tc.tile_pool(name="os", bufs=2))
        # PSUM: st(2 slots x 2 banks) + aux(2 slots x 1 bank) + av_a(1)
        # + av_b(1) = 8 banks.
        pp_st = ctx.enter_context(tc.tile_pool(name="pst", bufs=2, space="PSUM"))
        pp_av = ctx.enter_context(tc.tile_pool(name="pav", bufs=1, space="PSUM"))

        # ---- constants & weights (loaded once, outside the reps loop)
        bias_s = constp.tile([128, 6], f32, tag="bias", name="bias")
        bq_s = [bias_s[:, j:j + 1] for j in range(QC)]
        bk_s = bias_s[:, 4:5]
        bv_s = bias_s[:, 5:6]
        tri_s = constp.tile([128, 128], bf16, tag="tri", name="tri")
        ident = constp.tile([128, 128], bf16, tag="ident", name="ident")
        make_identity(nc, ident[:])
        ones_c = constp.tile([128, 1], bf16, tag="ones_c", name="ones_c")
        nc.vector.memset(ones_c[:], 1.0)


        wqt = wp.tile([128, KT * 512], bf16, tag="wqt", name="wqt")
        wkt = wp.tile([128, KT * 128], bf16, tag="wkt", name="wkt")
        wvt = wp.tile([128, KT * 128], bf16, tag="wvt", name="wvt")
        wot = wp.tile([128, 4 * T], bf16, tag="wot", name="wot")
        _loaded = set()

        def wq_l(kt, qc):
            return wqt[:, kt * 512 + qc * 128:kt * 512 + (qc + 1) * 128]

        def wk_l(kt):
            return wkt[:, kt * 128:(kt + 1) * 128]

        def wv_l(kt):
            return wvt[:, kt * 128:(kt + 1) * 128]

        def wo_l(h, ct):
            return wot[:, h * T + ct * 128:h * T + (ct + 1) * 128]

        def ensure_qkv_w():
            if "qkv" in _loaded:
                return
            _loaded.add("qkv")
            for q4 in range(4):
                k0 = q4 * 4
                nc.sync.dma_start(
                    wqt[:, k0 * 512:(k0 + 4) * 512].rearrange(
                        "p (k c) -> p k c", k=4
                    ),
                    wq_d[k0 * 128:(k0 + 4) * 128, :].rearrange(
                        "(k p) c -> p k c", p=128
                    ),
                )
            nc.sync.dma_start(
                wkt[:].rearrange("p (k c) -> p k c", k=KT),
                wk_d[:].rearrange("(k p) c -> p k c", p=128),
            )
            nc.sync.dma_start(
                wvt[:].rearrange("p (k c) -> p k c", k=KT),
                wv_d[:].rearrange("(k p) c -> p k c", p=128),
            )

        def ensure_wo():
            if "wo" in _loaded:
                return
            _loaded.add("wo")
            nc.sync.dma_start(
                wot[:].rearrange("p (h c) -> p h c", h=4),
                wo_d[:].rearrange("(h p) c -> p h c", p=128),
            )

        def ensure_consts():
            if "consts" in _loaded:
                return
            _loaded.add("consts")
            nc.sync.dma_start(bias_s[:], bias_d[:])
            nc.sync.dma_start(tri_s[:], tri_d[:])

        if reps != 1:
            ensure_qkv_w()
            ensure_consts()
            ensure_wo()

        qt_s = [pers.tile([128, T], bf16, tag=f"qt{j}", name=f"qt{j}") for j in range(QC)]
        kt_s = pers.tile([128, T], bf16, tag="kt", name="kt")
        v_s = [pers.tile([128, 130], bf16, tag=f"vs{k}", name=f"vs{k}") for k in range(KT)]

        def p1_dma(ts):
            """Strided loads for the 16 xt c-stripes of t-slice ts (4 chunks
            so the first sub-GEMM can start early; first chunk precedes the
            weight loads so the reps=1 cold start is shorter)."""
            xts = xtp.tile([128, KT * NT], bf16, tag="xt", name="xt")

            def chunk(q):
                k0 = q * 4
                nc.sync.dma_start(
                    xts[:, k0 * NT:(k0 + 4) * NT].rearrange(
                        "p (k c) -> p k c", k=4
                    ),
                    xt_d[k0 * 128:(k0 + 4) * 128, ts * NT:(ts + 1) * NT]
                    .rearrange("(k p) c -> p k c", p=128),
                )
            chunk(0)
            ensure_qkv_w()
            ensure_consts()
            for q in range(1, 4):
                chunk(q)
            return xts

        def p1_units(ts, xts):
            """Projection work for t-slice ts as (emit_fn, pe_us) fillers."""
            units = []

            def q_unit(qc, lo_kt, hi_kt, ps_box):
                def emit():
                    if ps_box[0] is None:
                        ps_box[0] = pp_st.tile(
                            [128, NT], f32, tag="aux", bufs=2, name=f"psq{qc}"
                        )
                    ps = ps_box[0]
                    for kt in range(lo_kt, hi_kt):
                        nc.tensor.matmul(
                            ps[:],
                            wq_l(kt, qc),
                            xts[:, kt * NT:(kt + 1) * NT],
                            start=(kt == 0),
                            stop=(kt == KT - 1),
                        )
                    if hi_kt == KT:
                        nc.vector.tensor_scalar_add(
                            qt_s[qc][:, ts * NT:(ts + 1) * NT], ps[:], bq_s[qc]
                        )
                return emit, (hi_kt - lo_kt) * 0.21

            for qc in range(QC):
                box = [None]
                units.append(q_unit(qc, 0, 8, box))
                units.append(q_unit(qc, 8, KT, box))

            def k_unit():
                ps_k = pp_av.tile([128, NT], f32, tag="av_a", name="av_a")
                for kt in range(KT):
                    nc.tensor.matmul(
                        ps_k[:], wk_l(kt), xts[:, kt * NT:(kt + 1) * NT],
                        start=(kt == 0), stop=(kt == KT - 1),
                    )
                nc.vector.tensor_scalar_add(
                    kt_s[:, ts * NT:(ts + 1) * NT], ps_k[:], bk_s
                )
            units.append((k_unit, 3.4))

            vtmp_box = [None]

            def v_unit():
                ps_v = pp_av.tile([128, NT], f32, tag="av_b", name="av_b")
                for kt in range(KT):
                    nc.tensor.matmul(
                        ps_v[:], wv_l(kt), xts[:, kt * NT:(kt + 1) * NT],
                        start=(kt == 0), stop=(kt == KT - 1),
                    )
                vtmp = tmpp.tile([128, NT], bf16, tag="vtmp", name="vtmp")
                nc.vector.tensor_scalar_add(vtmp[:], ps_v[:], bv_s)
                vtmp_box[0] = vtmp
            units.append((v_unit, 3.4))

            def vt_unit(r):
                def emit():
                    k4 = ts * 4 + r
                    vtmp = vtmp_box[0]
                    tp_ps = pp_st.tile(
                        [128, 128], bf16, tag="aux", bufs=2, name="vtp"
                    )
                    nc.tensor.transpose(
                        tp_ps[:], vtmp[:, r * 128:(r + 1) * 128], ident[:]
                    )
                    nc.vector.tensor_copy(v_s[k4][:, 0:64], tp_ps[:, 0:64])
                    nc.vector.tensor_copy(v_s[k4][:, 65:129], tp_ps[:, 64:128])
                    nc.vector.tensor_copy(v_s[k4][:, 64:65], ones_c[:])
                    nc.vector.tensor_copy(v_s[k4][:, 129:130], ones_c[:])
                return emit
            for r in range(4):
                units.append((vt_unit(r), 0.3))
            return units

        def oproj_units(qs, ots):
            """Output projection for q-slice qs as filler units; the 16
            c-blocks accumulate into one [128, KT*NT] tile, stored with a
            single strided DMA by the last unit."""
            units = []
            os_box = [None]

            def ct_unit(ct):
                def emit():
                    if os_box[0] is None:
                        os_box[0] = osp.tile(
                            [128, KT * NT], bf16, tag="os", name="os"
                        )
                    op_ps = pp_st.tile([128, NT], f32, tag="aux", bufs=2, name="op")
                    for h in range(4):
                        nc.tensor.matmul(
                            op_ps[:],
                            wo_l(h, ct),
                            ots[h][:],
                            start=(h == 0),
                            stop=(h == 3),
                        )
                    nc.vector.tensor_copy(
                        os_box[0][:, ct * NT:(ct + 1) * NT], op_ps[:]
                    )
                    if ct in (KT // 2 - 1, KT - 1):
                        half = KT // 2
                        h0 = 0 if ct < half else half
                        nc.sync.dma_start(
                            opt_d[h0 * 128:(h0 + half) * 128,
                                  qs * NT:(qs + 1) * NT]
                            .rearrange("(k p) c -> p k c", p=128),
                            os_box[0][:, h0 * NT:(h0 + half) * NT]
                            .rearrange("p (k c) -> p k c", k=half),
                        )
                return emit
            for ct in range(KT):
                units.append((ct_unit(ct), 0.9))
            return units

        def attn_slice(qs, fillers):
            """Attention for q-slice qs (needs p1 ts<=qs). Software-pipelined
            stripe stream across all 4 head-pairs; `fillers` are interleaved
            between stripes. Returns the normalized head outputs (ots)."""
            total_fill = sum(c for _, c in fillers)
            nkt = 4 * qs + 4
            n_stripes = nkt * QC
            fillers = list(fillers)
            filled = 0.0
            done_stripes = 0

            def maybe_fill(force=0):
                nonlocal filled
                target = total_fill * done_stripes / n_stripes
                n = 0
                while fillers and (filled < target or n < force):
                    emit, cost = fillers.pop(0)
                    emit()
                    filled += cost
                    n += 1

            avs = {}
            ots = []

            def emit_av(j, pt, kt, lo):
                if j not in avs:
                    avs[j] = (
                        pp_av.tile([128, NT], f32, tag="av_a", name="av_a"),
                        pp_av.tile([128, NT], f32, tag="av_b", name="av_b"),
                    )
                av_a, av_b = avs[j]
                nc.tensor.matmul(
                    av_a[0:65, lo:NT],
                    v_s[kt][:, 0:65],
                    pt[:, lo:NT],
                    start=(kt == 0),
                    stop=(kt == nkt - 1),
                )
                nc.tensor.matmul(
                    av_b[0:65, lo:NT],
                    v_s[kt][:, 65:130],
                    pt[:, NT + lo:2 * NT],
                    start=(kt == 0),
                    stop=(kt == nkt - 1),
                )

            def emit_norm(j):
                av_a, av_b = avs[j]
                rc2 = rcp.tile([1, 2 * NT], f32, tag="rc2", name="rc2")
                nc.vector.reciprocal(rc2[0:1, 0:NT], av_a[64:65, :])
                nc.vector.reciprocal(rc2[0:1, NT:2 * NT], av_b[64:65, :])
                bc = rcp.tile([64, 2 * NT], f32, tag="bc", name="bc")
                nc.gpsimd.partition_broadcast(bc[:, 0:NT], rc2[:, 0:NT])
                nc.gpsimd.partition_broadcast(bc[:, NT:2 * NT], rc2[:, NT:2 * NT])
                ot_j = otp.tile([128, NT], bf16, tag=f"ot{j}", name=f"ot{j}")
                nc.vector.tensor_mul(ot_j[0:64, :], av_a[0:64, :], bc[:, 0:NT])
                nc.vector.tensor_mul(
                    ot_j[64:128, :], av_b[0:64, :], bc[:, NT:2 * NT]
                )
                ots.append(ot_j)

            prev = None
            for j in range(QC):
                for kt in range(nkt):
                    r = kt - 4 * qs
                    lo = max(r, 0) * 128  # first unmasked q col in this slice
                    st_t = pp_st.tile([128, 2 * NT], f32, tag="st", name="st")
                    nc.tensor.matmul(
                        st_t[:, lo:NT],
                        kt_s[0:64, kt * 128:(kt + 1) * 128],
                        qt_s[j][0:64, qs * NT + lo:(qs + 1) * NT],
                        start=True,
                        stop=True,
                    )
                    nc.tensor.matmul(
                        st_t[:, NT + lo:2 * NT],
                        kt_s[64:128, kt * 128:(kt + 1) * 128],
                        qt_s[j][64:128, qs * NT + lo:(qs + 1) * NT],
                        start=True,
                        stop=True,
                    )
                    pt = ptp.tile([128, 2 * NT], bf16, tag="pt", name="pt")
                    if lo:
                        nc.scalar.activation(
                            pt[:].rearrange("p (s c) -> p s c", s=2)[:, :, lo:NT],
                            st_t[:].rearrange("p (s c) -> p s c", s=2)[:, :, lo:NT],
                            AF.Exp,
                            scale=0.125,
                        )
                    else:
                        nc.scalar.activation(pt[:], st_t[:], AF.Exp, scale=0.125)
                    if r >= 0:
                        # causal mask inside the diagonal 128x128 block
                        nc.vector.tensor_mul(
                            pt[:, lo:lo + 128], pt[:, lo:lo + 128], tri_s[:]
                        )
                        nc.vector.tensor_mul(
                            pt[:, NT + lo:NT + lo + 128],
                            pt[:, NT + lo:NT + lo + 128],
                            tri_s[:],
                        )
                    if prev is not None:
                        pj, ppt, pkt, plo = prev
                        emit_av(pj, ppt, pkt, plo)
                        boundary = pkt == nkt - 1
                        if boundary:
                            emit_norm(pj)
                        done_stripes += 1
                        maybe_fill(force=3 if boundary else 0)
                    prev = (j, pt, kt, lo)
            pj, ppt, pkt, plo = prev
            emit_av(pj, ppt, pkt, plo)
            emit_norm(pj)
            while fillers:
                emit, cost = fillers.pop(0)
                emit()
            return ots

        def body(_iv=None):
            do_oproj = "oproj" in sections
            # compact first projection slice; wo load queued after its DMAs
            xts = p1_dma(0)
            for emit, _ in p1_units(0, xts):
                emit()
            if do_oproj:
                ensure_wo()
            all_ots = []
            for ts in range(TS):
                fillers = []
                if ts >= 1 and do_oproj:
                    fillers += oproj_units(ts - 1, all_ots[ts - 1])
                if ts < TS - 1:
                    nxt = p1_dma(ts + 1)
                    fillers += p1_units(ts + 1, nxt)
                all_ots.append(attn_slice(ts, fillers))
            if do_oproj:
                for emit, _ in oproj_units(TS - 1, all_ots[TS - 1]):
                    emit()
            else:
                for j in range(QC):
                    os_t = osp.tile([128, NT], bf16, tag="os2", name="os2")
                    nc.vector.tensor_copy(os_t[:], all_ots[TS - 1][j][:])
                    nc.sync.dma_start(
                        opt_d[j * 128:(j + 1) * 128, 0:NT], os_t[:]
                    )

        if reps == 1:
            body()
        else:
            hints = (
                mybir.EngineType.PE,
                mybir.EngineType.DVE,
                mybir.EngineType.Activation,
            )
            with tc.For_i(0, reps, 1, hint_engines=hints) as _i:
                body(_i)

    nc.compile()
    _CACHE[key] = nc
    return nc


def _make_in_maps(inputs):
    x = np.asarray(inputs["x"], np.float32)
    Wq = np.asarray(inputs["Wq"], np.float32)
    bq = np.asarray(inputs["bq"], np.float32)
    Wk = np.asarray(inputs["Wk"], np.float32)
    bk = np.asarray(inputs["bk"], np.float32)
    Wv = np.asarray(inputs["Wv"], np.float32)
    bv = np.asarray(inputs["bv"], np.float32)
    Wo = np.asarray(inputs["Wo"], np.float32)

    try:
        import ml_dtypes

        bf = ml_dtypes.bfloat16

        def to_bf16(a):
            return np.ascontiguousarray(a.astype(bf))
    except ImportError:
        def to_bf16(a):
            # round-to-nearest-even fp32 -> bf16, stored as uint16
            u = np.ascontiguousarray(a, np.float32).view(np.uint32)
            rounded = (u + 0x7FFF + ((u >> 16) & 1)) >> 16
            return np.ascontiguousarray(rounded.astype(np.uint16))

    tri = np.triu(np.ones((128, 128), np.float32))
    in_maps = []
    for c in range(8):
        b, tp = c // 4, c % 4
        k0, k1 = 2 * tp, 2 * tp + 1
        qorder = np.concatenate(
            [
                np.r_[(4 * k + j) * D:(4 * k + j + 1) * D]
                for j in range(4)
                for k in (k0, k1)
            ]
        )
        kvorder = np.r_[k0 * D:(k0 + 1) * D, k1 * D:(k1 + 1) * D]
        bias = np.zeros((128, 6), np.float32)
        bias[:, 0:4] = bq[qorder].reshape(4, 128).T
        bias[:, 4] = bk[kvorder][0:128]
        bias[:, 5] = bv[kvorder][0:128]
        in_maps.append(
            {
                "xt": to_bf16(x[b].T),
                "wq": to_bf16(Wq[:, qorder]),
                "wk": to_bf16(Wk[:, kvorder]),
                "wv": to_bf16(Wv[:, kvorder]),
                "wo": to_bf16(Wo[qorder, :]),
                "bias": bias,
                "tri": to_bf16(tri),
            }
        )
    return in_maps


def _gather(results, bo):
    out = np.zeros((2, T, C), np.float32)
    for c in range(8):
        out[c // 4] += results[c]["opt"].astype(np.float32).T
    out += bo.astype(np.float32)
    return out


def kernel(**inputs):
    _ensure_path()
    from concourse.bass_utils import run_bass_kernel_spmd

    nc = _build(reps=1)
    in_maps = _make_in_maps(inputs)
    res = run_bass_kernel_spmd(nc, in_maps, list(range(8)))
    return _gather(res.results, np.asarray(inputs["bo"], np.float32))


def run_timed(inputs, reps, n_calls=3, sections=("p1", "attn", "oproj")):
    """Wall-clock the SPMD call at a given in-kernel rep count; returns
    (best_wall_seconds, outputs). Kernel time per rep is isolated by
    differencing two rep counts (data transfer is identical)."""
    import time

    _ensure_path()
    from concourse.bass_utils import run_bass_kernel_spmd

    nc = _build(reps=reps, sections=sections)
    in_maps = _make_in_maps(inputs)
    best = None
    res = None
    for _ in range(n_calls):
        t0 = time.time()
        res = run_bass_kernel_spmd(nc, in_maps, list(range(8)))
        dtm = time.time() - t0
        best = dtm if best is None else min(best, dtm)
    return best, _gather(res.results, np.asarray(inputs["bo"], np.float32))



# revision 39
# speedup vs baseline: 1.8741x; 1.8741x over previous
"""GQA forward kernel for 8 Trainium2 NeuronCores.

Problem: B=2, T=2048, C=2048, 32 Q heads / 8 KV heads, head_dim=64, causal.

Sharding: 2-way data parallel over batch x 4-way tensor parallel over KV-head
pairs. Each core handles one batch element and 2 KV heads (8 Q heads), computes
its slice of Q/K/V projections, causal attention, and a partial output
projection (transposed). Host sums the 4 partials per batch and adds bo.

Design (v7):
  - All matmul inputs bf16 (host converts); PSUM accumulation stays f32.
  - AV is TRANSPOSED vs v5: out[q, d] = sum_k P[k, q] * V[k, d], using the
    natural [k, q] layout of the exp output as the stationary operand and
    v_s[kt] as the moving operand. This uses all 128 output partitions per
    matmul (v5's [d, q] orientation filled only 65), halving AV's PE moving
    work. A ones column appended to each head's V gives the softmax
    denominator at out[q, 64]; normalization becomes per-partition
    tensor_scalar ops (no gpsimd partition_broadcast), followed by a PE
    transpose back to [d, q] for the output projection.
  - Attention q-slices are 256 wide (proj slices stay 512). The stream is
    SUPERSTRIPES: the two kt-tiles of a pair share one [128,1024]f32 st
    tile and a single full-width exp Activation (144 exps instead of 576
    stripe-halves). Within a superstripe, each head's scores live in their
    own PSUM bank: two row-packed (tile-positioned) matmuls must NOT write
    the same PSUM bank, or the per-bank accumulation state machine faults
    the device.
  - PSUM (8 banks): st 2x[128,1024] (2 banks each) + av 2x[128,260]
    (1 bank) + aux ring 2x1 bank for projection/oproj sub-GEMMs and
    transposes. The av tile holds 4 accumulation slices in one bank: one
    group per bank (start once on the first matmul, stop once on the last;
    in between, first touch of a lazily-zeroed byte overwrites).
  - Global deadline+deficit-paced scheduling: all projection strips
    (128-col, self-contained PSUM) and per-256-col oproj units live in one
    filler queue. Deadlines (global superstripe index of the first
    consumer) force availability; a deficit-proportional target paces the
    rest into the ACT-bound late groups, so the PE never idles against the
    back-loaded causal-attention exp stream. AV lags the scores stream by
    3 superstripes; norm transposes are deferred 2; causal masks run on the
    Pool engine (GPSIMD cannot touch PSUM, so the av-reading norm muls
    stay on DVE).
  - Cold start: slice-0 loads interleaved (xt column-halves, wkv packed
    early, wq in qc-halves) so k/v strips start at ~6us; slice-0 strips
    join the same filler queue.
  - DMA batching: >=512B contiguous runs everywhere (wk|wv packed into one
    (C,256) tensor -- 256-col runs pay a 2x DMA penalty), oproj stores
    every 4 c-tiles.
"""

import sys
import numpy as np

T = 2048
C = 2048
D = 64
NT = 512          # projection t-slice width
TS = T // NT      # 4 projection slices
KT = C // 128     # 16 contraction tiles
QC = 4            # local q-col tiles of 128 (512 local q cols)
AQ = 256          # attention q-slice width
AQS = T // AQ     # 8 attention q-slices

_CACHE = {}


def _ensure_path():
    for p in ("/opt/trn_rl_repo",):
        if p not in sys.path:
            sys.path.insert(0, p)


def _build(reps=1, sections=("p1", "attn", "oproj")):
    sections = tuple(sections)
    key = (reps, sections)
    if key in _CACHE:
        return _CACHE[key]
    _ensure_path()
    import concourse.mybir as mybir
    import concourse.bacc as bacc
    from concourse import tile
    from concourse.masks import make_identity
    from contextlib import ExitStack

    dt = mybir.dt
    f32 = dt.float32
    bf16 = dt.bfloat16
    AF = mybir.ActivationFunctionType

    nc = bacc.Bacc(None, target_bir_lowering=False)
    xt_d = nc.declare_dram_parameter("xt", (C, T), bf16, isOutput=False)
    wq_d = nc.declare_dram_parameter("wq", (C, 512), bf16, isOutput=False)
    wkv_d = nc.declare_dram_parameter("wkv", (C, 256), bf16, isOutput=False)
    wo_d = nc.declare_dram_parameter("wo", (512, C), bf16, isOutput=False)
    bias_d = nc.declare_dram_parameter("bias", (128, 6), f32, isOutput=False)
    tri_d = nc.declare_dram_parameter("tri", (128, 128), bf16, isOutput=False)
    opt_d = nc.declare_dram_parameter("opt", (C, T), bf16, isOutput=True)

    with tile.TileContext(nc) as tc, ExitStack() as ctx:
        constp = ctx.enter_context(tc.tile_pool(name="const", bufs=1))
        wp = ctx.enter_context(tc.tile_pool(name="w", bufs=1))
        pers = ctx.enter_context(tc.tile_pool(name="pers", bufs=1))
        otp = ctx.enter_context(tc.tile_pool(name="ot", bufs=3))
        xtp = ctx.enter_context(tc.tile_pool(name="xt", bufs=2))
        tmpp = ctx.enter_context(tc.tile_pool(name="tmp", bufs=2))
        ntp = ctx.enter_context(tc.tile_pool(name="nt", bufs=3))
        ptp = ctx.enter_context(tc.tile_pool(name="pt", bufs=8))
        rcp = ctx.enter_context(tc.tile_pool(name="rc", bufs=3))
        osp = ctx.enter_context(tc.tile_pool(name="os", bufs=2))
        # PSUM budget (8 banks): st 3x1 + av 3x1 + aux 2x1.
        pp_st = ctx.enter_context(tc.tile_pool(name="pst", bufs=3, space="PSUM"))
        pp_av = ctx.enter_context(tc.tile_pool(name="pav", bufs=3, space="PSUM"))
        pp_aux = ctx.enter_context(tc.tile_pool(name="paux", bufs=2, space="PSUM"))

        # ---- constants & weights (loaded once, outside the reps loop)
        bias_s = constp.tile([128, 6], f32, tag="bias", name="bias")
        bq_s = [bias_s[:, j:j + 1] for j in range(QC)]
        bk_s = bias_s[:, 4:5]
        bv_s = bias_s[:, 5:6]
        tri_s = constp.tile([128, 128], bf16, tag="tri", name="tri")
        ident = constp.tile([128, 128], bf16, tag="ident", name="ident")
        make_identity(nc, ident[:])
        ones_c = constp.tile([128, 1], bf16, tag="ones_c", name="ones_c")
        nc.vector.memset(ones_c[:], 1.0)
        tri2 = constp.tile([128, 256], bf16, tag="tri2", name="tri2")

        wqt = wp.tile([128, KT * 512], bf16, tag="wqt", name="wqt")
        wkvt = wp.tile([128, KT * 256], bf16, tag="wkvt", name="wkvt")
        wot = wp.tile([128, 4 * T], bf16, tag="wot", name="wot")
        _loaded = set()

        def wq_l(kt, qc):
            return wqt[:, kt * 512 + qc * 128:kt * 512 + (qc + 1) * 128]

        def wk_l(kt):
            return wkvt[:, kt * 256:kt * 256 + 128]

        def wv_l(kt):
            return wkvt[:, kt * 256 + 128:(kt + 1) * 256]

        def wo_l(h, ct):
            return wot[:, h * T + ct * 128:h * T + (ct + 1) * 128]

        def ensure_qkv_w():
            if "qkv" in _loaded:
                return
            _loaded.add("qkv")
            for q4 in range(4):
                k0 = q4 * 4
                nc.sync.dma_start(
                    wqt[:, k0 * 512:(k0 + 4) * 512].rearrange(
                        "p (k c) -> p k c", k=4
                    ),
                    wq_d[k0 * 128:(k0 + 4) * 128, :].rearrange(
                        "(k p) c -> p k c", p=128
                    ),
                )
            nc.sync.dma_start(
                wkvt[:].rearrange("p (k c) -> p k c", k=KT),
                wkv_d[:].rearrange("(k p) c -> p k c", p=128),
            )

        def ensure_wo():
            if "wo" in _loaded:
                return
            _loaded.add("wo")
            nc.sync.dma_start(
                wot[:].rearrange("p (h c) -> p h c", h=4),
                wo_d[:].rearrange("(h p) c -> p h c", p=128),
            )

        def ensure_consts():
            if "consts" in _loaded:
                return
            _loaded.add("consts")
            nc.sync.dma_start(bias_s[:], bias_d[:])
            nc.sync.dma_start(tri_s[:], tri_d[:])
            nc.vector.tensor_copy(tri2[:, 0:128], tri_s[:])
            nc.vector.tensor_copy(tri2[:, 128:256], tri_s[:])

        if reps != 1:
            ensure_qkv_w()
            ensure_consts()
            ensure_wo()

        qt_s = [pers.tile([128, T], bf16, tag=f"qt{j}", name=f"qt{j}") for j in range(QC)]
        kt_s = pers.tile([128, T], bf16, tag="kt", name="kt")
        v_s = [pers.tile([128, 130], bf16, tag=f"vs{k}", name=f"vs{k}") for k in range(KT)]

        def p1_dma(ts):
            """Strided loads for the 16 xt c-stripes of t-slice ts (4 chunks
            so the first sub-GEMM can start early; first chunk precedes the
            weight loads so the reps=1 cold start is shorter)."""
            xts = xtp.tile([128, KT * NT], bf16, tag="xt", name="xt")

            def chunk(q):
                k0 = q * 4
                nc.sync.dma_start(
                    xts[:, k0 * NT:(k0 + 4) * NT].rearrange(
                        "p (k c) -> p k c", k=4
                    ),
                    xt_d[k0 * 128:(k0 + 4) * 128, ts * NT:(ts + 1) * NT]
                    .rearrange("(k p) c -> p k c", p=128),
                )
            chunk(0)
            ensure_qkv_w()
            ensure_consts()
            for q in range(1, 4):
                chunk(q)
            return xts

        # ---- global stripe indexing: stream is (qs, j, kt) for qs 0..7,
        # j 0..3, kt 0..2qs+1; groups pair q-slices (2g, 2g+1).
        def nkt_of(qs):
            return 2 * qs + 2

        def base_of(qs):
            return 4 * (qs * qs + qs)

        def idx_of(qs, j, kt):
            return base_of(qs) + j * nkt_of(qs) + kt

        def p1_dma0():
            """Cold-start loads for t-slice 0, interleaved so the first
            attention stripes can start ~8us earlier: xt in two column
            halves (512B runs, full DMA rate), wkv early (k/v strips are
            cheap starters), wq in qc-halves."""
            xts = xtp.tile([128, KT * NT], bf16, tag="xt", name="xt")

            def xt_half(h):
                nc.sync.dma_start(
                    xts[:].rearrange("p (k c) -> p k c", k=KT)[
                        :, :, h * 256:(h + 1) * 256
                    ],
                    xt_d[:, 0:NT].rearrange("(k p) c -> p k c", p=128)[
                        :, :, h * 256:(h + 1) * 256
                    ],
                )

            if "qkv" in _loaded:
                xt_half(0)
                xt_half(1)
                return xts
            _loaded.add("qkv")

            def wq_half(h):
                nc.sync.dma_start(
                    wqt[:].rearrange("p (k c) -> p k c", k=KT)[
                        :, :, h * 256:(h + 1) * 256
                    ],
                    wq_d[:, h * 256:(h + 1) * 256].rearrange(
                        "(k p) c -> p k c", p=128
                    ),
                )

            xt_half(0)
            nc.sync.dma_start(
                wkvt[:].rearrange("p (k c) -> p k c", k=KT),
                wkv_d[:].rearrange("(k p) c -> p k c", p=128),
            )
            wq_half(0)
            ensure_consts()
            xt_half(1)
            wq_half(1)
            return xts

        def p1_units(ts, xts):
            """Projection work for t-slice ts as fine-grained fillers:
            (emit, cost_us, deadline) per 128-wide column strip,
            self-contained in PSUM. Deadlines are the global stripe index of
            the first consumer, so the pacer can defer strips deep into
            group ts itself (attention is back-loaded; late groups need the
            PE work)."""
            units = []

            def q_unit(qc, r):
                def emit():
                    ps = pp_aux.tile([128, 128], f32, tag="aux", name=f"psq{qc}")
                    c0 = r * 128
                    for kt in range(KT):
                        nc.tensor.matmul(
                            ps[:],
                            wq_l(kt, qc),
                            xts[:, kt * NT + c0:kt * NT + c0 + 128],
                            start=(kt == 0),
                            stop=(kt == KT - 1),
                        )
                    nc.vector.tensor_scalar_add(
                        qt_s[qc][:, ts * NT + c0:ts * NT + c0 + 128],
                        ps[:], bq_s[qc],
                    )
                return emit

            def k_unit(r):
                def emit():
                    ps_k = pp_aux.tile([128, 128], f32, tag="aux", name="psk")
                    c0 = r * 128
                    for kt in range(KT):
                        nc.tensor.matmul(
                            ps_k[:], wk_l(kt),
                            xts[:, kt * NT + c0:kt * NT + c0 + 128],
                            start=(kt == 0), stop=(kt == KT - 1),
                        )
                    nc.vector.tensor_scalar_add(
                        kt_s[:, ts * NT + c0:ts * NT + c0 + 128], ps_k[:], bk_s
                    )
                return emit

            def v_unit(r, box):
                # proj half: PE matmuls + DVE bias-add into an SBUF strip
                def emit():
                    ps_v = pp_aux.tile([128, 128], f32, tag="aux", name="psv")
                    c0 = r * 128
                    for kt in range(KT):
                        nc.tensor.matmul(
                            ps_v[:], wv_l(kt),
                            xts[:, kt * NT + c0:kt * NT + c0 + 128],
                            start=(kt == 0), stop=(kt == KT - 1),
                        )
                    vtmp = tmpp.tile([128, 128], bf16, tag="vtmp", name="vtmp")
                    nc.vector.tensor_scalar_add(vtmp[:], ps_v[:], bv_s)
                    box[0] = vtmp
                return emit

            def vt_unit(r, box):
                # transpose half: scheduled a few stripes after v_unit so the
                # PE never waits on the DVE bias-add
                def emit():
                    k4 = ts * 4 + r
                    vtmp = box[0]
                    tp_ps = pp_aux.tile([128, 128], bf16, tag="aux", name="vtp")
                    nc.tensor.transpose(tp_ps[:], vtmp[:], ident[:])
                    nc.vector.tensor_copy(v_s[k4][:, 0:64], tp_ps[:, 0:64])
                    nc.vector.tensor_copy(v_s[k4][:, 65:129], tp_ps[:, 64:128])
                    nc.vector.tensor_copy(v_s[k4][:, 64:65], ones_c[:])
                    nc.vector.tensor_copy(v_s[k4][:, 129:130], ones_c[:])
                return emit

            for r in range(4):
                # k/v strip r first consumed at kt-tile 4ts+r, i.e. in
                # q-slice (4ts+r)//2 (scores lead AV, so k's index bounds
                # both).
                dl = idx_of((4 * ts + r) // 2, 0, 4 * ts + r)
                box = [None]
                units.append((k_unit(r), 0.87, dl))
                units.append((v_unit(r, box), 0.87, dl - 4))
                units.append((vt_unit(r, box), 0.08, dl))
            for r in range(4):
                # q strip r (cols ts*512+r*128) first consumed by scores of
                # q-slice 2ts + r//2 at j=qc.
                for qc in range(QC):
                    dl = idx_of(2 * ts + r // 2, qc, 0)
                    units.append((q_unit(qc, r), 0.87, dl))
            units.sort(key=lambda u: u[2])
            return units

        def oproj_units(qs, ots):
            """Output projection for attention q-slice qs (256 cols of opt):
            16 ct filler units of [128,256], accumulating into one
            [128, KT*256] tile, stored with two strided DMAs."""
            units = []
            os_box = [None]

            def ct_unit(ct):
                def emit():
                    if os_box[0] is None:
                        os_box[0] = osp.tile(
                            [128, KT * AQ], bf16, tag="os", name="os"
                        )
                    op_ps = pp_aux.tile([128, AQ], f32, tag="aux", name="op")
                    q0 = (qs % 2) * AQ
                    for h in range(4):
                        nc.tensor.matmul(
                            op_ps[:],
                            wo_l(h, ct),
                            ots[h][:, q0:q0 + AQ],
                            start=(h == 0),
                            stop=(h == 3),
                        )
                    nc.vector.tensor_copy(
                        os_box[0][:, ct * AQ:(ct + 1) * AQ], op_ps[:]
                    )
                    if ct % 4 == 3:
                        h0 = ct - 3
                        nc.sync.dma_start(
                            opt_d[h0 * 128:(h0 + 4) * 128,
                                  qs * AQ:(qs + 1) * AQ]
                            .rearrange("(k p) c -> p k c", p=128),
                            os_box[0][:, h0 * AQ:(h0 + 4) * AQ]
                            .rearrange("p (k c) -> p k c", k=4),
                        )
                return emit
            for ct in range(KT):
                units.append((ct_unit(ct), 0.45))
            return units

        def body(_iv=None):
            do_oproj = "oproj" in sections
            # ---- cold start: slice-0 loads; its projection strips join the
            # deadline-paced filler queue like every other slice
            xts0 = p1_dma0()
            if do_oproj:
                ensure_wo()
            if "attn" not in sections:
                for emit, _c, _d in p1_units(0, xts0):
                    emit()
                for ts in range(1, TS):
                    nxt = p1_dma(ts)
                    for emit, _c, _d in p1_units(ts, nxt):
                        emit()
                os_t = osp.tile([128, NT], bf16, tag="os2", name="os2")
                nc.vector.tensor_copy(os_t[:], qt_s[0][:, 0:NT])
                nc.sync.dma_start(opt_d[0:128, 0:NT], os_t[:])
                return

            # ---- per-stripe deficit estimates (us): exp on ACT is the
            # stripe-rate limiter; the PE needs roughly ACT-PE of filler per
            # stripe to stay busy.
            import os as _os
            _qs_max = int(_os.environ.get("KATTN_QS_MAX", str(AQS)))
            _j_max = int(_os.environ.get("KATTN_J_MAX", str(QC)))
            stream = [
                (qs, j, kt)
                for qs in range(min(AQS, _qs_max))
                for j in range(min(QC, _j_max))
                for kt in range(nkt_of(qs))
            ]
            n_total = len(stream)
            dcum = []
            acc = 0.0
            for (qs, j, kt) in stream:
                w = 256 - max(kt - 2 * qs, 0) * 128
                nav = 2 * (2 if kt <= 2 * qs else 1)
                act = (2 * w * 0.833 + 185) / 1000.0
                pe = ((2 * w + nav * 65) * 0.4167 + (2 + nav) * 2.2) / 1000.0
                acc += max(act - pe, 0.06)
                dcum.append(acc)
            d_total = acc
            f_total = (
                4 * (16 * 0.87 + 4 * 0.87 + 4 * 0.87 + 4 * 0.08)
                + (7 * 16 * 0.45 if do_oproj else 0)
            )

            # ---- unified filler queue
            hard = list(p1_units(0, xts0))   # (emit, cost, deadline)
            soft = []   # (emit, cost) FIFO, no deadline (oproj)
            state = {"filled": 0.0}
            MARGIN = 7

            excess = max(f_total - d_total, 0.0)

            def fill_for(i):
                while hard and hard[0][2] <= i + MARGIN:
                    emit, cost, _ = hard.pop(0)
                    emit()
                    state["filled"] += cost
                # cover the per-stripe ACT deficit, plus a uniform share of
                # the excess PE work (otherwise PE-rich early groups idle
                # against the st ring while fillers are held for later)
                target = dcum[i] + excess * (i + 1) / n_total
                while (soft or hard) and state["filled"] < target:
                    if soft:
                        emit, cost = soft.pop(0)
                    else:
                        emit, cost, _ = hard.pop(0)
                    emit()
                    state["filled"] += cost

            # ---- attention state
            LAG = 6
            avs = {}
            reserve = []
            soft_pending = []
            pending = []
            normq = []  # (due_stripe, emit_pe_part)
            norms_left = {qs: 8 for qs in range(AQS)}
            group_ots = {}

            def ot_of(g, j):
                key = (g, j)
                if key not in group_ots:
                    group_ots[key] = otp.tile(
                        [128, NT], bf16, tag=f"ot{j}", name=f"ot{j}"
                    )
                return group_ots[key]

            def emit_norm_dve(qs, j, s, av, i):
                rc = rcp.tile([128, 2], f32, tag="rc", name="rc")
                nc.vector.reciprocal(
                    rc[:, 0:1], av[:, 130 * s + 64:130 * s + 65]
                )
                nc.vector.reciprocal(
                    rc[:, 1:2], av[:, 130 * s + 129:130 * s + 130]
                )
                tmp = ntp.tile([128, 128], bf16, tag="ntmp", name="ntmp")
                # must be DVE: GPSIMD cannot access PSUM (walrus verifier)
                nc.vector.tensor_scalar_mul(
                    tmp[:, 0:64], av[:, 130 * s:130 * s + 64], rc[:, 0:1]
                )
                nc.vector.tensor_scalar_mul(
                    tmp[:, 64:128], av[:, 130 * s + 65:130 * s + 129],
                    rc[:, 1:2],
                )

                def pe_part():
                    tp = pp_aux.tile([128, 128], bf16, tag="aux", name="ntp")
                    nc.tensor.transpose(tp[:], tmp[:], ident[:])
                    nc.vector.tensor_copy(
                        ot_of(qs // 2, j)[:, (qs % 2) * 256 + s * 128:
                                          (qs % 2) * 256 + s * 128 + 128],
                        tp[:],
                    )
                    norms_left[qs] -= 1
                    if norms_left[qs] == 0 and do_oproj:
                        units = oproj_units(
                            qs, [ot_of(qs // 2, jj) for jj in range(QC)]
                        )
                        if qs == AQS - 2:
                            # hold a few units back to fill the PE during the
                            # end-of-stream norm/AV drain
                            reserve.extend(units[-6:])
                            units = units[:-6]
                        # 2 superstripes of delay so the DVE ot copies have
                        # drained before the first oproj matmul needs them
                        soft_pending.append((i + 2, units))
                normq.append((i + 3, pe_part))

            def emit_av(qs, j, kt, pt, i):
                if (qs, j) not in avs:
                    avs[(qs, j)] = pp_av.tile(
                        [128, 4 * 65], f32, tag="av", name="av"
                    )
                av = avs[(qs, j)]
                # PSUM zero regions are bank-sized: only one accumulation
                # group may be open per bank. The first matmul (kt=0, s=0,
                # h=0) starts the group for the whole bank (lazy-zeroing it);
                # every other slice accumulates into it (first touch of a
                # pending-zero byte overwrites); only the very last matmul
                # of the (qs, j) tile stops the group.
                for s in range(2):
                    if kt > 2 * qs + s:
                        continue
                    for h in range(2):
                        nc.tensor.matmul(
                            av[:, 130 * s + 65 * h:130 * s + 65 * h + 65],
                            pt[:, 256 * h + 128 * s:256 * h + 128 * s + 128],
                            v_s[kt][:, 65 * h:65 * h + 65],
                            start=(kt == 0 and s == 0 and h == 0),
                            stop=(s == 1 and h == 1 and kt == 2 * qs + 1),
                        )
                if kt == 2 * qs + 1:
                    # norms read the PSUM bank: legal only once the bank's
                    # accumulation group is closed (single group per bank)
                    emit_norm_dve(qs, j, 0, av, i)
                    emit_norm_dve(qs, j, 1, av, i)

            def run_norms(i):
                while normq and normq[0][0] <= i:
                    normq.pop(0)[1]()
                while soft_pending and soft_pending[0][0] <= i:
                    soft.extend(soft_pending.pop(0)[1])

            # ---- the stream
            for i, (qs, j, kt) in enumerate(stream):
                if kt == 0 and j == 0 and qs % 2 == 0 and not _os.environ.get("KATTN_NO_P1"):
                    g = qs // 2
                    if g + 1 < TS:
                        nxt = p1_dma(g + 1)
                        hard.extend(p1_units(g + 1, nxt))
                        hard.sort(key=lambda u: u[2])
                run_norms(i)
                fill_for(i)
                r = kt - 2 * qs
                lo = max(r, 0) * 128
                st_t = pp_st.tile([128, 512], f32, tag="st", name="st")
                nc.tensor.matmul(
                    st_t[:, lo:256],
                    kt_s[0:64, kt * 128:(kt + 1) * 128],
                    qt_s[j][0:64, qs * AQ + lo:(qs + 1) * AQ],
                    start=True,
                    stop=True,
                )
                nc.tensor.matmul(
                    st_t[:, 256 + lo:512],
                    kt_s[64:128, kt * 128:(kt + 1) * 128],
                    qt_s[j][64:128, qs * AQ + lo:(qs + 1) * AQ],
                    start=True,
                    stop=True,
                )
                pt = ptp.tile([128, 512], bf16, tag="pt", name="pt")
                if lo:
                    nc.scalar.activation(
                        pt[:].rearrange("p (s c) -> p s c", s=2)[:, :, lo:256],
                        st_t[:].rearrange("p (s c) -> p s c", s=2)[:, :, lo:256],
                        AF.Exp,
                        scale=0.125,
                    )
                else:
                    nc.scalar.activation(pt[:], st_t[:], AF.Exp, scale=0.125)
                if r >= 0:
                    # causal mask inside the diagonal 128x128 block (on the
                    # otherwise-idle Pool engine: keeps the exp->mask->AV
                    # chain off the busy DVE queue)
                    nc.gpsimd.tensor_mul(
                        pt[:, lo:lo + 128], pt[:, lo:lo + 128], tri_s[:]
                    )
                    nc.gpsimd.tensor_mul(
                        pt[:, 256 + lo:256 + lo + 128],
                        pt[:, 256 + lo:256 + lo + 128],
                        tri_s[:],
                    )
                pending.append((qs, j, kt, pt))
                if len(pending) > LAG:
                    pqs, pj, pkt, ppt = pending.pop(0)
                    if "av" in sections or "oproj" in sections:
                        emit_av(pqs, pj, pkt, ppt, i)

            # ---- drain: interleave reserve fillers with the tail AV/norm
            # dependency chains so the PE has work while DVE catches up
            for step, (pqs, pj, pkt, ppt) in enumerate(pending):
                if "av" in sections or "oproj" in sections:
                    emit_av(pqs, pj, pkt, ppt, n_total + step)
                run_norms(n_total + step)
                if reserve:
                    reserve.pop(0)[0]()
            for step in range(LAG, LAG + 4):
                run_norms(n_total + step)
                if reserve:
                    reserve.pop(0)[0]()
            run_norms(n_total + LAG + 16)
            while reserve:
                reserve.pop(0)[0]()
            while hard:
                emit, cost, _ = hard.pop(0)
                emit()
            for _due, units in soft_pending:
                soft.extend(units)
            soft_pending.clear()
            while soft:
                emit, cost = soft.pop(0)
                emit()
            if not do_oproj:
                for j in range(QC):
                    os_t = osp.tile([128, NT], bf16, tag="os2", name="os2")
                    if "av" not in sections:
                        nc.vector.tensor_copy(os_t[:], qt_s[j][:, 0:NT])
                        nc.sync.dma_start(
                            opt_d[j * 128:(j + 1) * 128, 0:NT], os_t[:]
                        )
                        continue
                    nc.vector.tensor_copy(os_t[:], ot_of(TS - 1, j)[:])
                    nc.sync.dma_start(
                        opt_d[j * 128:(j + 1) * 128, 0:NT], os_t[:]
                    )

        if reps == 1:
            body()
        else:
            hints = (
                mybir.EngineType.PE,
                mybir.EngineType.DVE,
                mybir.EngineType.Activation,
            )
            with tc.For_i(0, reps, 1, hint_engines=hints) as _i:
                body(_i)

    nc.compile()
    _CACHE[key] = nc
    return nc


def _make_in_maps(inputs):
    x = np.asarray(inputs["x"], np.float32)
    Wq = np.asarray(inputs["Wq"], np.float32)
    bq = np.asarray(inputs["bq"], np.float32)
    Wk = np.asarray(inputs["Wk"], np.float32)
    bk = np.asarray(inputs["bk"], np.float32)
    Wv = np.asarray(inputs["Wv"], np.float32)
    bv = np.asarray(inputs["bv"], np.float32)
    Wo = np.asarray(inputs["Wo"], np.float32)

    try:
        import ml_dtypes

        bf = ml_dtypes.bfloat16

        def to_bf16(a):
            return np.ascontiguousarray(a.astype(bf))
    except ImportError:
        def to_bf16(a):
            # round-to-nearest-even fp32 -> bf16, stored as uint16
            u = np.ascontiguousarray(a, np.float32).view(np.uint32)
            rounded = (u + 0x7FFF + ((u >> 16) & 1)) >> 16
            return np.ascontiguousarray(rounded.astype(np.uint16))

    tri = np.triu(np.ones((128, 128), np.float32))
    in_maps = []
    for c in range(8):
        b, tp = c // 4, c % 4
        k0, k1 = 2 * tp, 2 * tp + 1
        qorder = np.concatenate(
            [
                np.r_[(4 * k + j) * D:(4 * k + j + 1) * D]
                for j in range(4)
                for k in (k0, k1)
            ]
        )
        kvorder = np.r_[k0 * D:(k0 + 1) * D, k1 * D:(k1 + 1) * D]
        bias = np.zeros((128, 6), np.float32)
        bias[:, 0:4] = bq[qorder].reshape(4, 128).T
        bias[:, 4] = bk[kvorder][0:128]
        bias[:, 5] = bv[kvorder][0:128]
        in_maps.append(
            {
                "xt": to_bf16(x[b].T),
                "wq": to_bf16(Wq[:, qorder]),
                "wkv": to_bf16(
                    np.concatenate([Wk[:, kvorder], Wv[:, kvorder]], axis=1)
                ),
                "wo": to_bf16(Wo[qorder, :]),
                "bias": bias,
                "tri": to_bf16(tri),
            }
        )
    return in_maps


def _gather(results, bo):
    out = np.zeros((2, T, C), np.float32)
    for c in range(8):
        out[c // 4] += results[c]["opt"].astype(np.float32).T
    out += bo.astype(np.float32)
    return out


def kernel(**inputs):
    _ensure_path()
    from concourse.bass_utils import run_bass_kernel_spmd

    nc = _build(reps=1)
    in_maps = _make_in_maps(inputs)
    res = run_bass_kernel_spmd(nc, in_maps, list(range(8)))
    return _gather(res.results, np.asarray(inputs["bo"], np.float32))


def run_timed(inputs, reps, n_calls=3, sections=("p1", "attn", "oproj")):
    """Wall-clock the SPMD call at a given in-kernel rep count; returns
    (best_wall_seconds, outputs). Kernel time per rep is isolated by
    differencing two rep counts (data transfer is identical)."""
    import time

    _ensure_path()
    from concourse.bass_utils import run_bass_kernel_spmd

    nc = _build(reps=reps, sections=sections)
    in_maps = _make_in_maps(inputs)
    best = None
    res = None
    for _ in range(n_calls):
        t0 = time.time()
        res = run_bass_kernel_spmd(nc, in_maps, list(range(8)))
        dtm = time.time() - t0
        best = dtm if best is None else min(best, dtm)
    return best, _gather(res.results, np.asarray(inputs["bo"], np.float32))
